# revision 1
# baseline (speedup 1.0000x reference)
"""Trainium2 Bass kernel for nn_InterleavedHiddenMarkovChain_47261820125822.

Math: in the reference, the dense (N,N) score matrix M (N = S*S*K = 4608)
is -inf except where the full state tuple of x_old equals x_new's (the
`same` mask compares all K components), so each column has exactly K=2
finite entries and the scan collapses exactly.  With
g[c,s,y] = choice_l[c] + trans_l[c,s,s] + emis_l[c,s,y]:

    beta_0[s0,s1] = prior_l[0,s0] + prior_l[1,s1] + LSE_c(choice_l)
    beta_t = beta_{t-1} + h_t,  h_t[s0,s1] = LSE(g[0,s0,y_t], g[1,s1,y_t])
    answer = LSE_{s0,s1} beta_T

This is bitwise-equal math to the dense scan (the -inf entries contribute
exact zeros to each logsumexp).  Using LSE(a,b) = b + log1p(exp(a-b))
(|a-b| < 40 here, so no overflow), sum_t splits into sum_t G1[s1,t]
(separable; one ones-matmul) + sum_t log1p(exp(G0[s0,t]-G1[s1,t])) — a
single (T=64 partitions) x (48*48 free) fused elementwise pass.

Sharding across the 8 cores: the collapsed problem is ~150K flops, far
below per-core fixed overheads, so the sharding-hint's row-sharded psum
scheme would be pure loss.  We replicate: all 8 cores run the identical
NEFF SPMD (the hint is advisory; "distribute as you see fit"), and the
host takes core 0's scalar.  All floating-point work happens on-device;
the host only reshapes inputs, builds the one-hot of ys (index prep),
and constant tensors (identity / ones).
"""

import numpy as np

import concourse.bass as bass
import concourse.bacc as bacc
import concourse.mybir as mybir
from concourse import tile
from concourse.bass_utils import run_bass_kernel_spmd

F32 = mybir.dt.float32
AF = mybir.ActivationFunctionType
AX = mybir.AxisListType
OP = mybir.AluOpType

K, S, A, T = 2, 48, 64, 64
CS = K * S          # 96 (c,s) rows
N2 = S * S          # 2304
N_CORES = 8

_CACHED_NC = None


def _build_nc():
    nc = bacc.Bacc("TRN2", target_bir_lowering=False, debug=False)

    tr = nc.dram_tensor("trans", [CS, S], F32, kind="ExternalInput")
    em = nc.dram_tensor("emis", [CS, A], F32, kind="ExternalInput")
    pr = nc.dram_tensor("prior", [K, S], F32, kind="ExternalInput")
    ch = nc.dram_tensor("choice", [1, K], F32, kind="ExternalInput")
    yoh = nc.dram_tensor("yoh", [A, T], F32, kind="ExternalInput")
    id96 = nc.dram_tensor("id96", [CS, CS], F32, kind="ExternalInput")
    ones64 = nc.dram_tensor("ones64", [T, 1], F32, kind="ExternalInput")
    out_d = nc.dram_tensor("out", [1, 1], F32, kind="ExternalOutput")

    with tile.TileContext(nc) as tc:
        with (
            tc.tile_pool(name="sb", bufs=1) as sb,
            tc.tile_pool(name="ps", bufs=1, space="PSUM") as ps,
        ):
            def load(name, dram, shape):
                t = sb.tile(shape, F32, tag=name)
                nc.sync.dma_start(t[:], dram[:, :])
                return t

            TT = load("TT", tr, [CS, S])
            EM = load("EM", em, [CS, A])
            CH = load("CH", ch, [1, K])
            # prior rows as separate partition-0 tiles (engine APs may only
            # start at partitions {0,32,64,96})
            PR0 = sb.tile([1, S], F32, tag="PR0")
            nc.sync.dma_start(PR0[:], pr[0:1, :])
            PR1 = sb.tile([1, S], F32, tag="PR1")
            nc.sync.dma_start(PR1[:], pr[1:2, :])
            YOH = load("YOH", yoh, [A, T])
            ID = load("ID", id96, [CS, CS])
            ON = load("ON", ones64, [T, 1])
            # diagonal transition[c,s,s]: per c-block a stride-(S+1) walk
            DG = sb.tile([CS, 1], F32, tag="DG")
            nc.sync.dma_start(
                DG[:], bass.AP(tr, 0, [[S * S, K], [S + 1, S], [1, 1]]))

            def row_lse(x_ap, P, W, name):
                """per-partition logsumexp over the free axis -> (P,1)"""
                nm = sb.tile([P, 1], F32, tag=f"nm_{name}")
                nc.vector.tensor_reduce(nm[:], x_ap, axis=AX.X, op=OP.max,
                                        negate=True)
                e = sb.tile([P, W], F32, tag=f"e_{name}")
                nc.scalar.activation(e[:], x_ap, AF.Exp, bias=nm[:])
                s = sb.tile([P, 1], F32, tag=f"s_{name}")
                nc.vector.tensor_reduce(s[:], e[:], axis=AX.X, op=OP.add)
                l = sb.tile([P, 1], F32, tag=f"l_{name}")
                nc.scalar.activation(l[:], s[:], AF.Ln)
                lse = sb.tile([P, 1], F32, tag=f"lse_{name}")
                nc.vector.tensor_sub(lse[:], l[:], nm[:])
                return lse

            lseT = row_lse(TT[:], CS, S, "T")
            lseE = row_lse(EM[:], CS, A, "E")

            lseC = row_lse(CH[:], 1, K, "C")
            CHL = sb.tile([1, K], F32, tag="CHL")
            nc.vector.tensor_scalar_sub(CHL[:], CH[:], lseC[:])
            cL = row_lse(CHL[:], 1, K, "C2")      # LSE_c choice_l  (~0)

            lseP0 = row_lse(PR0[:], 1, S, "P0")
            PRL0 = sb.tile([1, S], F32, tag="PRL0")
            nc.vector.tensor_scalar_sub(PRL0[:], PR0[:], lseP0[:])
            lseP1 = row_lse(PR1[:], 1, S, "P1")
            PRL1 = sb.tile([1, S], F32, tag="PRL1")
            nc.vector.tensor_scalar_sub(PRL1[:], PR1[:], lseP1[:])

            # choice_l along free as (1, 96) for the accumulate-matmul:
            # CROW[0, c*S+s] = choice_l[c]
            CROW = sb.tile([1, CS], F32, tag="CROW")
            nc.vector.tensor_copy(
                CROW[:].rearrange("p (c s) -> p c s", c=K, s=S),
                CHL[:].unsqueeze(2).broadcast_to([1, K, S]))
            ONR = sb.tile([1, T], F32, tag="ONR")
            nc.vector.memset(ONR[:], 1.0)

            # per-(c,s) additive term: diag - lseT - lseE
            t0 = sb.tile([CS, 1], F32, tag="t0")
            nc.vector.tensor_sub(t0[:], DG[:], lseT[:])
            PCOL = sb.tile([CS, 1], F32, tag="PCOL")
            nc.vector.tensor_sub(PCOL[:], t0[:], lseE[:])

            # G_full[(c,s), a] = emission + per-row constant
            GF = sb.tile([CS, A], F32, tag="GF")
            nc.vector.tensor_scalar_add(GF[:], EM[:], PCOL[:])

            # transpose to (a, (c,s)), then gather columns by ys via the
            # one-hot matmul; choice_l rides in as a rank-1 accumulate
            GFT_p = ps.tile([A, CS], F32, tag="ps_small")
            nc.tensor.transpose(GFT_p[:], GF[:], ID[:])
            GFT = sb.tile([A, CS], F32, tag="GFT")
            nc.vector.tensor_copy(GFT[:], GFT_p[:])
            GT_p = ps.tile([T, CS], F32, tag="ps_small")
            nc.tensor.matmul(GT_p[:], YOH[:], GFT[:], start=True, stop=False)
            nc.tensor.matmul(GT_p[:], ONR[:], CROW[:], start=False, stop=True)
            GTs = sb.tile([T, CS], F32, tag="GTs")
            nc.vector.tensor_copy(GTs[:], GT_p[:])

            # d[t, s0, s1] = G0[t,s0] - G1[t,s1]  via stride-0 broadcasts
            u0 = GTs[:, 0:S]
            u1 = GTs[:, S:CS]
            u0b = u0.unsqueeze(2).broadcast_to([T, S, S])
            u1b = u1.unsqueeze(1).broadcast_to([T, S, S])
            D = sb.tile([T, S, S], F32, tag="D")
            nc.vector.tensor_sub(D[:], u0b, u1b)
            Df = D[:].rearrange("p a b -> p (a b)")
            EX = sb.tile([T, N2], F32, tag="EX")
            nc.scalar.activation(EX[:], Df, AF.Exp)
            SP = sb.tile([T, N2], F32, tag="SP")
            nc.scalar.activation(SP[:], EX[:], AF.Ln, bias=1.0)

            # NL[s0,s1] = sum_t SP  (contract partition dim with ones)
            NL_p = ps.tile([1, N2], F32, tag="NL_p")
            for j0 in range(0, N2, 512):
                w = min(512, N2 - j0)
                nc.tensor.matmul(NL_p[:, j0:j0 + w], ON[:], SP[:, j0:j0 + w])
            R1_p = ps.tile([1, S], F32, tag="ps_small")
            nc.tensor.matmul(R1_p[:], ON[:], u1)

            NLs = sb.tile([1, N2], F32, tag="NLs")
            nc.vector.tensor_copy(NLs[:], NL_p[:])
            R1s = sb.tile([1, S], F32, tag="R1s")
            nc.vector.tensor_copy(R1s[:], R1_p[:])

            # total[s0,s1] = NL + R1[s1] + prior_l[0,s0] + prior_l[1,s1] + cL
            T1 = sb.tile([1, S, S], F32, tag="T1")
            nc.vector.tensor_add(
                T1[:], NLs[:].rearrange("p (a b) -> p a b", a=S, b=S),
                R1s[:].unsqueeze(1).broadcast_to([1, S, S]))
            T2 = sb.tile([1, S, S], F32, tag="T2")
            nc.vector.tensor_add(
                T2[:], T1[:], PRL0[:].unsqueeze(2).broadcast_to([1, S, S]))
            T3 = sb.tile([1, S, S], F32, tag="T3")
            nc.vector.tensor_add(
                T3[:], T2[:], PRL1[:].unsqueeze(1).broadcast_to([1, S, S]))
            T4 = sb.tile([1, N2], F32, tag="T4")
            nc.vector.tensor_scalar_add(
                T4[:], T3[:].rearrange("p a b -> p (a b)"), cL[:])

            # final logsumexp over all 2304 entries
            fin = row_lse(T4[:], 1, N2, "F")
            nc.sync.dma_start(out_d[:, :], fin[:])

    nc.compile()
    return nc


def _host_inputs(ys, transition, emission, choice, prior):
    ys = np.asarray(ys).astype(np.int64)
    yoh = (ys[None, :] == np.arange(A)[:, None]).astype(np.float32)
    return {
        "trans": np.ascontiguousarray(
            np.asarray(transition, np.float32).reshape(CS, S)),
        "emis": np.ascontiguousarray(
            np.asarray(emission, np.float32).reshape(CS, A)),
        "prior": np.ascontiguousarray(np.asarray(prior, np.float32)),
        "choice": np.asarray(choice, np.float32).reshape(1, K),
        "yoh": yoh,
        "id96": np.eye(CS, dtype=np.float32),
        "ones64": np.ones((T, 1), np.float32),
    }


def kernel(ys, transition, emission, choice, prior):
    global _CACHED_NC
    if _CACHED_NC is None:
        _CACHED_NC = _build_nc()
    in_map = _host_inputs(ys, transition, emission, choice, prior)
    in_maps = [dict(in_map) for _ in range(N_CORES)]
    res = run_bass_kernel_spmd(_CACHED_NC, in_maps,
                               core_ids=list(range(N_CORES)))
    return np.float32(res.results[0]["out"][0, 0]).reshape(())



# revision 8
# speedup vs baseline: 1.8836x; 1.8836x over previous
"""Trainium2 Bass kernel for nn_InterleavedHiddenMarkovChain_47261820125822.

Math: in the reference, the dense (N,N) score matrix M (N = S*S*K = 4608)
is -inf except where the full state tuple of x_old equals x_new's (the
`same` mask compares all K components), so each column has exactly K=2
finite entries and the scan collapses exactly.  With probability-domain
tables F_c[s,a] = softmax(choice)[c] * softmax(trans[c,s,:])[s] *
softmax(emis[c,s,:])[a] and u_c = ln F_c:

    beta[s0,s1] = p0[s0] + p1[s1] + sum_t LSE(u0[s0,y_t], u1[s1,y_t])
    answer      = LSE_{s0,s1} beta

Using LSE(a,b) = b + log1p(exp(a-b)) and grouping the t-sum by symbol
counts cnt[a] = #{t: y_t = a} (integer prep on host):

    beta = p0[s0] + (p1 + sum_a cnt[a] u1[s1,a])[s1]
         + sum_a cnt[a] * log1p(F0[s0,a] / F1[s1,a])

so the only big work is one (A=64 part, S*S=2304 free) elementwise
multiply G0[a,s0] * (1/F1)[a,s1] (both free-dim broadcasts), one
log1p activation, and a cnt-weighted column-sum matmul (fp32r).

Perf notes vs the previous version (65.7us):
 - One manual InstLoadActFuncSet of the joint exp+ln table kills all
   activation-table thrash (was 13 x 1283ns of reloads).
 - The bias/prior terms are built as a (48,48) grid via PE outer
   product and folded in a (18,128) layout; the final LSE runs on 18
   partitions with gpsimd partition_all_reduce -- no more (1,2304)
   single-lane vector ops (~20us in the old tail).
 - Softmaxes are computed by division (vector.reciprocal), batched in
   two wide tiles; exp uses fused accum_out for the row sums.
 - The per-(c,s) choice*diag scale rides the PE transpose as a scaled
   diagonal instead of an identity.

Sharding across the 8 cores: the collapsed problem is ~150K flops, far
below per-core fixed overheads, so the sharding-hint's row-sharded psum
scheme would be pure loss.  We replicate: all 8 cores run the identical
NEFF SPMD, and the host takes core 0's scalar.  Host does only integer
index prep (symbol counts, diag mask) and reshapes; all float math is
on-device.
"""

import numpy as np

import concourse.bass as bass
import concourse.bacc as bacc
import concourse.mybir as mybir
from concourse import tile
from concourse.bass_isa import ReduceOp
from concourse.bass_utils import run_bass_kernel_spmd

F32 = mybir.dt.float32
F32R = mybir.dt.float32r
AF = mybir.ActivationFunctionType
AX = mybir.AxisListType
OP = mybir.AluOpType

K, S, A, T = 2, 48, 64, 64
N2 = S * S          # 2304
N_CORES = 8
NEG = -30.0         # pad fill; exp(NEG - max) == 0 to fp32
# act_info.json table index of natural_log_exp_and_others (exp + ln in
# one piecewise-poly table -> a single ACT_TABLE_LOAD serves the kernel)
ACT_TABLE_EXP_LN = 6

_CACHED_NC = None


def _build_nc():
    nc = bacc.Bacc("TRN2", target_bir_lowering=False, debug=False)

    tr = nc.dram_tensor("trans", [K * S, S], F32, kind="ExternalInput")
    em = nc.dram_tensor("emis", [K * S, A], F32, kind="ExternalInput")
    pr = nc.dram_tensor("prior", [K, S], F32, kind="ExternalInput")
    ch = nc.dram_tensor("choice", [1, K], F32, kind="ExternalInput")
    cnt_d = nc.dram_tensor("cnt", [A, 1], F32, kind="ExternalInput")
    im_d = nc.dram_tensor("imask", [128, 128], F32, kind="ExternalInput")
    out_d = nc.dram_tensor("out", [1, 1], F32, kind="ExternalOutput")

    with tile.TileContext(nc) as tc:
        with (
            tc.tile_pool(name="sb", bufs=1) as sb,
            tc.tile_pool(name="ps", bufs=1, space="PSUM") as ps,
        ):
            # single activation table load (exp+ln); nothing to wait on,
            # runs while input DMAs are in flight
            nc.scalar.add_instruction(mybir.InstLoadActFuncSet(
                name=nc.get_next_instruction_name(),
                act_func_set_id=ACT_TABLE_EXP_LN, ins=[], outs=[]))

            # ---- tiles (CS64 layout: row c*64+s; c-blocks at partitions
            # 0 and 64 so engine AP starts stay in {0,32,64,96}) ----
            TRX = sb.tile([128, S], F32, tag="TRX")
            EMX = sb.tile([128, A], F32, tag="EMX")
            DG = sb.tile([128, 1], F32, tag="DG")
            IM = sb.tile([128, 128], F32, tag="IM")
            PR0 = sb.tile([1, S], F32, tag="PR0")
            PR1 = sb.tile([1, S], F32, tag="PR1")
            CHX = sb.tile([1, K], F32, tag="CHX")
            CNT = sb.tile([A, 1], F32, tag="CNT")

            nc.gpsimd.memset(TRX[:], NEG)
            nc.gpsimd.memset(EMX[:], NEG)
            nc.gpsimd.memset(DG[:], NEG)

            # input DMAs, spread across engine queues
            nc.sync.dma_start(TRX[0:S, :], tr[0:S, :])
            nc.sync.dma_start(TRX[64:64 + S, :], tr[S:2 * S, :])
            # diagonal transition[c,s,s]: stride-(S+1) walk per c block
            nc.sync.dma_start(
                DG[0:S, :], bass.AP(tr, 0, [[S + 1, S], [1, 1]]))
            nc.sync.dma_start(
                DG[64:64 + S, :], bass.AP(tr, S * S, [[S + 1, S], [1, 1]]))
            nc.sync.dma_start(CHX[:], ch[:, :])
            nc.gpsimd.dma_start(EMX[0:S, :], em[0:S, :])
            nc.gpsimd.dma_start(EMX[64:64 + S, :], em[S:2 * S, :])
            nc.gpsimd.dma_start(IM[:], im_d[:, :])
            nc.gpsimd.dma_start(CNT[:], cnt_d[:, :])
            nc.scalar.dma_start(PR0[:], pr[0:1, :])
            nc.scalar.dma_start(PR1[:], pr[1:2, :])

            # constants
            ONES48 = sb.tile([1, S], F32, tag="ONES48")
            nc.vector.memset(ONES48[:], 1.0)
            ONE1 = sb.tile([1, 1], F32, tag="ONE1")
            nc.vector.memset(ONE1[:], 1.0)

            # ---- normalizers (probability domain, division-style) ----
            # emission rows -> E_prob[(c,s), a]
            nmE = sb.tile([128, 1], F32, tag="nmE")
            nc.vector.tensor_reduce(nmE[:], EMX[:], axis=AX.X, op=OP.max,
                                    negate=True)
            EMe = sb.tile([128, A], F32, tag="EMe")
            ZE = sb.tile([128, 1], F32, tag="ZE")
            nc.scalar.activation(EMe[:], EMX[:], AF.Exp, bias=nmE[:],
                                 accum_out=ZE[:])
            ZEr = sb.tile([128, 1], F32, tag="ZEr")
            nc.vector.reciprocal(ZEr[:], ZE[:])
            EP = sb.tile([128, A], F32, tag="EP")
            nc.vector.tensor_scalar_mul(EP[:], EMe[:], ZEr[:])

            # transition rows: only row-sum and diagonal needed
            nmT = sb.tile([128, 1], F32, tag="nmT")
            nc.vector.tensor_reduce(nmT[:], TRX[:], axis=AX.X, op=OP.max,
                                    negate=True)
            TJ = sb.tile([128, S], F32, tag="TJ")
            ZT = sb.tile([128, 1], F32, tag="ZT")
            nc.scalar.activation(TJ[:], TRX[:], AF.Exp, bias=nmT[:],
                                 accum_out=ZT[:])
            ZTr = sb.tile([128, 1], F32, tag="ZTr")
            nc.vector.reciprocal(ZTr[:], ZT[:])
            dex = sb.tile([128, 1], F32, tag="dex")
            nc.scalar.activation(dex[:], DG[:], AF.Exp, bias=nmT[:])
            dpr = sb.tile([128, 1], F32, tag="dpr")
            nc.vector.tensor_mul(dpr[:], dex[:], ZTr[:])

            # choice -> c_prob[1,2], broadcast to all partitions
            nmC = sb.tile([1, 1], F32, tag="nmC")
            nc.vector.tensor_reduce(nmC[:], CHX[:], axis=AX.X, op=OP.max,
                                    negate=True)
            CHe = sb.tile([1, K], F32, tag="CHe")
            ZC = sb.tile([1, 1], F32, tag="ZC")
            nc.scalar.activation(CHe[:], CHX[:], AF.Exp, bias=nmC[:],
                                 accum_out=ZC[:])
            ZCr = sb.tile([1, 1], F32, tag="ZCr")
            nc.vector.reciprocal(ZCr[:], ZC[:])
            CPr = sb.tile([1, K], F32, tag="CPr")
            nc.vector.tensor_scalar_mul(CPr[:], CHe[:], ZCr[:])
            CB = sb.tile([128, K], F32, tag="CB")
            nc.gpsimd.partition_broadcast(CB[:], CPr[:])

            # cd[(c,s)] = c_prob[c] * d_prob[(c,s)] as scaled diagonal
            cd = sb.tile([128, 1], F32, tag="cd")
            nc.vector.memset(cd[:], 0.0)
            nc.vector.tensor_mul(cd[0:S, :], dpr[0:S, :], CB[0:S, 0:1])
            nc.vector.tensor_mul(cd[64:64 + S, :], dpr[64:64 + S, :],
                                 CB[64:64 + S, 1:2])
            D128 = sb.tile([128, 128], F32, tag="D128")
            nc.vector.tensor_mul(D128[:], IM[:],
                                 cd[:].broadcast_to([128, 128]))

            # EPT[a, (c,s)] = F_c(s, a): transpose E_prob scaled by cd
            EPT_p = ps.tile([A, 128], F32, tag="ps_ept")
            nc.tensor.matmul(EPT_p[:], EP[:], D128[:], start=True, stop=True)
            EPT = sb.tile([A, 128], F32, tag="EPT")
            nc.vector.tensor_copy(EPT[:], EPT_p[:])

            # 1/F1 and ln F1 tables (free-dim slices of EPT)
            H1T = sb.tile([A, S], F32, tag="H1T")
            nc.vector.reciprocal(H1T[:], EPT[:, 64:64 + S])
            LF1 = sb.tile([A, S], F32, tag="LF1")
            nc.scalar.activation(LF1[:], EPT[:, 64:64 + S], AF.Ln)

            # ---- prior log rows and bias grid B ----
            def prior_row(src, name):
                nm = sb.tile([1, 1], F32, tag=f"nm{name}")
                nc.vector.tensor_reduce(nm[:], src[:], axis=AX.X, op=OP.max,
                                        negate=True)
                e = sb.tile([1, S], F32, tag=f"e{name}")
                z = sb.tile([1, 1], F32, tag=f"z{name}")
                nc.scalar.activation(e[:], src[:], AF.Exp, bias=nm[:],
                                     accum_out=z[:])
                lz = sb.tile([1, 1], F32, tag=f"lz{name}")
                nc.scalar.activation(lz[:], z[:], AF.Ln)
                row = sb.tile([1, S], F32, tag=f"row{name}")
                nc.vector.scalar_tensor_tensor(
                    row[:], src[:], nm[:], lz[:].broadcast_to([1, S]),
                    op0=OP.add, op1=OP.subtract)
                return row

            p0row = prior_row(PR0, "P0")
            p1row = prior_row(PR1, "P1")

            # R1[s1] = sum_a cnt[a] * ln F1[s1, a]
            R1_p = ps.tile([1, S], F32, tag="ps_r1")
            nc.tensor.matmul(R1_p[:], CNT[:], LF1[:], start=True, stop=True)
            q1 = sb.tile([1, S], F32, tag="q1")
            nc.vector.tensor_add(q1[:], p1row[:], R1_p[:])

            # B2s[s0, s1] = p0[s0] + q1[s1]
            p0c_p = ps.tile([S, 1], F32, tag="ps_p0")
            nc.tensor.matmul(p0c_p[:], p0row[:], ONE1[:], start=True,
                             stop=True)
            p0c = sb.tile([S, 1], F32, tag="p0c")
            nc.vector.tensor_copy(p0c[:], p0c_p[:])
            B2d_p = ps.tile([S, S], F32, tag="ps_b2d")
            nc.tensor.matmul(B2d_p[:], ONES48[:], q1[:], start=True,
                             stop=True)
            B2s = sb.tile([S, S], F32, tag="B2s")
            nc.vector.tensor_scalar_add(B2s[:], B2d_p[:], p0c[:])
            B16 = sb.tile([16, 144], F32, tag="B16")
            nc.sync.dma_start(B16[:], B2s[:])

            # ---- big phase: W[a, s0, s1] = F0[s0,a] / F1[s1,a] ----
            G0 = EPT[:, 0:S]
            W = sb.tile([A, S, S], F32, tag="W")
            SPX = sb.tile([A, N2], F32R, tag="SPX")
            NCH = 3
            SCH = S // NCH
            for j in range(NCH):
                s_lo = j * SCH
                nc.vector.tensor_mul(
                    W[:, s_lo:s_lo + SCH, :],
                    G0[:, s_lo:s_lo + SCH].unsqueeze(2)
                      .broadcast_to([A, SCH, S]),
                    H1T[:].unsqueeze(1).broadcast_to([A, SCH, S]))
                nc.scalar.activation(
                    SPX[:, s_lo * S:(s_lo + SCH) * S],
                    W[:, s_lo:s_lo + SCH, :].rearrange("p a b -> p (a b)"),
                    AF.Ln, bias=1.0)

            # NL row: cnt-weighted column sums (fp32r matmuls, 512-col
            # psum-bank chunks, ping-pong banks, copies spread over
            # scalar/gpsimd/vector)
            NLS = sb.tile([1, N2], F32, tag="NLS")
            CNTR = sb.tile([A, 1], F32R, tag="CNTR")
            nc.gpsimd.tensor_copy(CNTR[:], CNT[:])
            NL_pa = ps.tile([1, 512], F32, tag="ps_nla")
            NL_pb = ps.tile([1, 512], F32, tag="ps_nlb")
            NL_p = [NL_pa, NL_pb]
            for c in range(5):
                lo = 512 * c
                w = min(512, N2 - lo)
                dst = NL_p[c % 2]
                nc.tensor.matmul(dst[:, 0:w], CNTR[:],
                                 SPX[:, lo:lo + w],
                                 start=True, stop=True)
                if c % 2 == 0:
                    nc.scalar.copy(NLS[:, lo:lo + w], dst[:, 0:w])
                else:
                    nc.vector.tensor_copy(NLS[:, lo:lo + w], dst[:, 0:w])
            NL16 = sb.tile([16, 144], F32, tag="NL16")
            nc.sync.dma_start(NL16[:], NLS[:])

            # ---- tail: LSE over beta = NL + B on 18 partitions ----
            T16 = sb.tile([16, 144], F32, tag="T16")
            nc.vector.tensor_add(T16[:], NL16[:], B16[:])
            M1 = sb.tile([16, 1], F32, tag="M1")
            nc.vector.tensor_reduce(M1[:], T16[:], axis=AX.X, op=OP.max)
            Mg = sb.tile([16, 1], F32, tag="Mg")
            nc.gpsimd.partition_all_reduce(Mg[:], M1[:], 16, ReduceOp.max)
            Mn = sb.tile([16, 1], F32, tag="Mn")
            nc.vector.tensor_scalar_mul(Mn[:], Mg[:], -1.0)
            EX16 = sb.tile([16, 144], F32, tag="EX16")
            S1 = sb.tile([16, 1], F32, tag="S1")
            nc.scalar.activation(EX16[:], T16[:], AF.Exp, bias=Mn[:],
                                 accum_out=S1[:])
            Sg = sb.tile([16, 1], F32, tag="Sg")
            nc.gpsimd.partition_all_reduce(Sg[:], S1[:], 16, ReduceOp.add)
            lnS = sb.tile([1, 1], F32, tag="lnS")
            nc.scalar.activation(lnS[:], Sg[0:1, :], AF.Ln)
            ans = sb.tile([1, 1], F32, tag="ans")
            nc.vector.tensor_add(ans[:], lnS[:], Mg[0:1, :])
            nc.sync.dma_start(out_d[:, :], ans[:])

    nc.compile()
    return nc


def _host_inputs(ys, transition, emission, choice, prior):
    ys = np.asarray(ys).astype(np.int64)
    cnt = np.bincount(ys, minlength=A).astype(np.float32).reshape(A, 1)
    imask = np.zeros((128, 128), np.float32)
    for c in range(K):
        for s in range(S):
            r = c * 64 + s
            imask[r, r] = 1.0
    return {
        "trans": np.ascontiguousarray(
            np.asarray(transition, np.float32).reshape(K * S, S)),
        "emis": np.ascontiguousarray(
            np.asarray(emission, np.float32).reshape(K * S, A)),
        "prior": np.ascontiguousarray(np.asarray(prior, np.float32)),
        "choice": np.asarray(choice, np.float32).reshape(1, K),
        "cnt": cnt,
        "imask": imask,
    }


def kernel(ys, transition, emission, choice, prior):
    global _CACHED_NC
    if _CACHED_NC is None:
        _CACHED_NC = _build_nc()
    in_map = _host_inputs(ys, transition, emission, choice, prior)
    in_maps = [dict(in_map) for _ in range(N_CORES)]
    res = run_bass_kernel_spmd(_CACHED_NC, in_maps,
                               core_ids=list(range(N_CORES)))
    return np.float32(res.results[0]["out"][0, 0]).reshape(())
